# revision 3
# baseline (speedup 1.0000x reference)
"""SpMM (GCN layer) kernel v2 for 8 TRN2 NeuronCores.

out[i] = sum_{e: row[e]==i} vals[e] * embeds[col[e]]     (N=100000, E=3.2M, d=32)

v2 vs baseline: the bottleneck is SWDGE descriptor generation on GpSimd
(~4ns/descriptor, 1 desc per gathered slot), so the design minimizes
slot count:
  - Single logical chunk: the table is packed as fp16 QUADS (4 nodes
    = 128 fp16 = 256B payload per descriptor), so a 15-bit index covers
    all 100000/4 = 25000 quads.  Per-edge sub-node selection is folded
    into the vals multiply (vals4[slot, s] = val if col%4==s else 0).
  - Exact-K slots: per core, rows are sorted by degree (host-side
    permutation, undone for free on output) and each 128-row q-block
    gets K_q = max degree in block slots per row.  ~404K slots/core vs
    603K padded + overflow batches in the baseline; no overflow path.
  - The window schedule (gather instruction sizes / reduce shapes) is
    shared across cores: K_q = max over the 8 cores at each block rank.
"""

import sys

if "/opt/trn_rl_repo" not in sys.path:
    sys.path.insert(0, "/opt/trn_rl_repo")

import numpy as np

import concourse.bass as bass
import concourse.tile as tile
from concourse import bacc, mybir, bass_utils
from concourse import ap_utils
from concourse.bass import round_up_to_multiple, exact_div

# ---------------- problem geometry (hardcoded) ----------------
N_NODES = 100000
N_EDGES = 3200000
D = 32
NC = 8
RPC = N_NODES // NC            # rows per core = 12500
RPAD = 12544                   # 98 * 128
QROWS = RPAD // 128            # 98
NQUAD = N_NODES // 4           # 25000 quads, int16-indexable
PAYLOAD = 128                  # fp16 elems per gathered quad (256B)
TSTRIDE = 256                  # fp16 elems per table row (512B, 2x payload)
TCOL_MAX = 192                 # max gather-window columns
NUM_Q = 4                      # swdge queues used (desc-gen parallelizes per queue)
SINGLE_PACKET = False

_cache = {}


def _dma_gather_raw(gp, out_ap, in_ap, idxs_ap, num_idxs, num_idxs_reg,
                    elem_size, elem_step, queue_num=0, single_packet=False):
    """dma_gather minus the blanket elem_size_bytes%256 assert (payload
    size is free; only the descriptor stride needs 256B granularity)."""
    assert idxs_ap.dtype == mybir.dt.int16
    assert in_ap.dtype == out_ap.dtype
    assert in_ap.space == bass.MemorySpace.DRAM
    assert idxs_ap.space == bass.MemorySpace.SBUF
    assert out_ap.space == bass.MemorySpace.SBUF
    assert ap_utils.ap_is_contiguous(in_ap.ap[1:])
    assert ap_utils.ap_is_contiguous(out_ap.ap[1:])
    assert ap_utils.ap_is_contiguous(idxs_ap.ap[1:])
    assert in_ap.ap[-1][1] == out_ap.ap[-1][1] == elem_size
    assert out_ap.ap[0][1] * out_ap.ap[1][1] == round_up_to_multiple(num_idxs, 128)
    assert in_ap.ap[0][0] == elem_step
    stride_bytes = elem_step * mybir.dt.size(in_ap.dtype)
    stride_bytes_256 = exact_div(stride_bytes, 256)
    assert stride_bytes_256 < 256
    _in_ap = gp.lower_ap_dma(in_ap, for_custom_bir_dma=True)
    _idxs_ap = gp.lower_ap(idxs_ap)
    _out_ap = gp.lower_ap(out_ap)
    return gp.add_instruction(
        mybir.InstDMAGatherAnt(
            name=gp.bass.get_next_instruction_name(),
            ins=[*_in_ap, _idxs_ap, gp.lower_val_access(gp.to_reg(num_idxs_reg))],
            outs=[_out_ap],
            transpose=False,
            num_idxs=num_idxs,
            elem_size=elem_size,
            stride_bytes_256=stride_bytes_256,
            gen_mode=0,
            single_packet=single_packet,
            queue_num=queue_num,
            sbuf_tokens_per_rank=0,
            sbuf_free_dim_per_rank=0,
            sbuf_free_dim_pad_per_rank=0,
            sbuf_byte_offset=0,
        )
    )


def _schedule(K_q):
    """Shared window schedule from the per-rank max-K profile.
    Returns list of (qa, qe, k0, k1, accumulate) and total_cols."""
    windows = []
    q = 0
    while q < QROWS:
        K = int(K_q[q])
        if K == 0:
            break
        qb = q
        while qb < QROWS and K_q[qb] == K:
            qb += 1
        if K > TCOL_MAX:
            for qq in range(q, qb):
                k0 = 0
                while k0 < K:
                    k1 = min(k0 + TCOL_MAX, K)
                    windows.append((qq, qq + 1, k0, k1, k0 > 0))
                    k0 = k1
        else:
            nq_max = max(1, TCOL_MAX // K)
            qa = q
            while qa < qb:
                qe = min(qa + nq_max, qb)
                windows.append((qa, qe, 0, K, False))
                qa = qe
        q = qb
    total_cols = sum((qe - qa) * (k1 - k0) for qa, qe, k0, k1, _ in windows)
    return windows, total_cols


def _build_module(windows, total_cols):
    nc = bacc.Bacc("TRN2", target_bir_lowering=False, num_swdge_queues=NUM_Q)
    f32, f16, i16 = mybir.dt.float32, mybir.dt.float16, mybir.dt.int16

    tab = nc.dram_tensor("tab", [NQUAD, TSTRIDE], f16, kind="ExternalInput")
    idxs = nc.dram_tensor("idxs", [128, total_cols * 8], i16, kind="ExternalInput")
    vals = nc.dram_tensor("vals", [128, total_cols * 4], f16, kind="ExternalInput")
    y = nc.dram_tensor("y", [RPAD, D], f32, kind="ExternalOutput")

    with tile.TileContext(nc) as tc:
        with tc.tile_pool(name="acc", bufs=1) as accp, \
             tc.tile_pool(name="work", bufs=2) as wp, \
             nc.allow_low_precision(reason="f16 4-term sub-node fold; "
                                    "final k-reduce accumulates in f32"):
            out_acc = accp.tile([128, QROWS * D], f32)
            nc.vector.memset(out_acc[:], 0.0)

            # warmup: a tiny gather triggers the Q7 library load + ring
            # setup while the first real idx tile is still loading
            wi_t = accp.tile([128, 8], i16)
            wg_t = accp.tile([128, PAYLOAD], f16)
            nc.vector.memset(wi_t[:], 0)
            _dma_gather_raw(
                nc.gpsimd,
                wg_t[:].rearrange("p (c e) -> p c e", e=PAYLOAD),
                tab[:, :PAYLOAD], wi_t[:],
                num_idxs=128, num_idxs_reg=128,
                elem_size=PAYLOAD, elem_step=TSTRIDE,
                queue_num=0, single_packet=SINGLE_PACKET)

            def write_y(qa, qe):
                nc.sync.dma_start(
                    out=y[qa * 128:qe * 128, :].rearrange(
                        "(q p) d -> p q d", p=128),
                    in_=out_acc[:, qa * D:qe * D].rearrange(
                        "p (q d) -> p q d", d=D))

            # a window is the final writer of its q-range iff no later
            # window touches the same q-blocks
            final_writer = []
            for w, (qa, qe, _, _, _) in enumerate(windows):
                later = windows[w + 1:]
                final_writer.append(
                    all(qe <= la or qa >= le for la, le, _, _, _ in later))

            off = 0
            for w, (qa, qe, k0, k1, acc_flag) in enumerate(windows):
                nq = qe - qa
                kk = k1 - k0
                cols = nq * kk
                nidx = cols * 128
                idx_t = wp.tile([128, cols * 8], i16, tag="idx")
                val_t = wp.tile([128, cols * 4], f16, tag="val")
                g_t = wp.tile([128, cols * PAYLOAD], f16, tag="g")
                nc.sync.dma_start(
                    out=idx_t[:], in_=idxs[:, off * 8:(off + cols) * 8])
                nc.scalar.dma_start(
                    out=val_t[:], in_=vals[:, off * 4:(off + cols) * 4])
                # split the gather across all SWDGE queues; desc-gen for the
                # queues runs concurrently on the Q7 (one burst serves all)
                base = cols // NUM_Q
                rem = cols % NUM_Q
                c0 = 0
                for qn in range(NUM_Q):
                    cw = base + (1 if qn < rem else 0)
                    if cw == 0:
                        continue
                    c1 = c0 + cw
                    _dma_gather_raw(
                        nc.gpsimd,
                        g_t[:, c0 * PAYLOAD:c1 * PAYLOAD].rearrange(
                            "p (c e) -> p c e", e=PAYLOAD),
                        tab[:, :PAYLOAD], idx_t[:, c0 * 8:c1 * 8],
                        num_idxs=cw * 128, num_idxs_reg=cw * 128,
                        elem_size=PAYLOAD, elem_step=TSTRIDE,
                        queue_num=qn, single_packet=SINGLE_PACKET)
                    c0 = c1
                # scale by vals4 (selects the sub-node); table quads are
                # interleaved [d, s] so the innermost axis is contiguous
                # (s, step 1) on every operand -> DVE 2x_1P mode.  The
                # s-fold runs as tensor_tensor halves (tensor_reduce over
                # the packed axis cannot use 2x).
                gv = g_t[:].rearrange("p (c d s) -> p c d s", s=4, d=D)
                vb = val_t[:].rearrange("p (c s) -> p c s", s=4) \
                    .unsqueeze(2).broadcast_to((128, cols, D, 4))
                nc.vector.tensor_tensor(out=gv, in0=gv, in1=vb,
                                        op=mybir.AluOpType.mult)
                nc.vector.tensor_tensor(
                    out=gv[:, :, :, 0:2], in0=gv[:, :, :, 0:2],
                    in1=gv[:, :, :, 2:4], op=mybir.AluOpType.add)
                r1_t = wp.tile([128, cols * D], f16, tag="r1")
                nc.vector.tensor_tensor(
                    out=r1_t[:].rearrange("p (c d) -> p c d", d=D),
                    in0=gv[:, :, :, 0], in1=gv[:, :, :, 1],
                    op=mybir.AluOpType.add)
                # reduce over k -> [p, nq, d]
                rin = r1_t[:].rearrange("p (q k d) -> p q d k", k=kk, d=D)
                if not acc_flag:
                    nc.vector.tensor_reduce(
                        out=out_acc[:, qa * D:qe * D].rearrange(
                            "p (q d) -> p q d", d=D),
                        in_=rin, axis=mybir.AxisListType.X,
                        op=mybir.AluOpType.add)
                else:
                    red_t = wp.tile([128, nq * D], f32, tag="red")
                    nc.vector.tensor_reduce(
                        out=red_t[:].rearrange("p (q d) -> p q d", d=D),
                        in_=rin, axis=mybir.AxisListType.X,
                        op=mybir.AluOpType.add)
                    nc.vector.tensor_tensor(
                        out=out_acc[:, qa * D:qe * D],
                        in0=out_acc[:, qa * D:qe * D],
                        in1=red_t[:], op=mybir.AluOpType.add)
                if final_writer[w]:
                    write_y(qa, qe)
                off += cols

            covered = set()
            for qa, qe, _, _, _ in windows:
                covered.update(range(qa, qe))
            q = 0
            while q < QROWS:
                if q in covered:
                    q += 1
                    continue
                qe = q
                while qe < QROWS and qe not in covered:
                    qe += 1
                write_y(q, qe)
                q = qe

    nc.finalize()
    return nc


def _preprocess(adj_row, adj_col, adj_vals, embeds):
    order = np.argsort(adj_row, kind="stable")
    rows = np.ascontiguousarray(adj_row[order]).astype(np.int64)
    cols = np.ascontiguousarray(adj_col[order]).astype(np.int64)
    vals = np.ascontiguousarray(adj_vals[order]).astype(np.float32)
    bounds = np.searchsorted(rows, np.arange(NC + 1) * RPC)

    # fp16 quad table, 512B row stride (payload in first 256B)
    tab = np.zeros((NQUAD, TSTRIDE), np.float16)
    tab[:, :PAYLOAD] = (embeds.astype(np.float16)
                        .reshape(NQUAD, 4, D)
                        .transpose(0, 2, 1)
                        .reshape(NQUAD, PAYLOAD))

    # per-core degree-sorted rows; shared K profile
    per_core = []
    K_prof = np.zeros(QROWS, np.int64)
    for m in range(NC):
        s, e = bounds[m], bounds[m + 1]
        rl = rows[s:e] - m * RPC
        deg = np.bincount(rl, minlength=RPAD)
        perm = np.argsort(-deg, kind="stable")   # perm[rank] = local row
        rank_of = np.empty(RPAD, np.int64)
        rank_of[perm] = np.arange(RPAD)
        K_prof = np.maximum(K_prof,
                            deg[perm].reshape(QROWS, 128).max(axis=1))
        per_core.append((s, e, rl, perm, rank_of))

    windows, total_cols = _schedule(K_prof)

    # map (q, k) -> global column
    colmap = np.full((QROWS, int(K_prof.max())), -1, np.int64)
    off = 0
    for qa, qe, k0, k1, _ in windows:
        kk = k1 - k0
        for qq in range(qa, qe):
            colmap[qq, k0:k1] = off + (qq - qa) * kk + np.arange(kk)
        off += (qe - qa) * kk
    assert off == total_cols

    rng = np.random.default_rng(12345)
    in_maps = []
    for m in range(NC):
        s, e, rl, perm, rank_of = per_core[m]
        cc = cols[s:e]
        vv = vals[s:e]
        rr = rank_of[rl]
        o2 = np.argsort(rr, kind="stable")
        rr, cc2, vv2 = rr[o2], cc[o2], vv[o2]
        uniq, starts, counts = np.unique(rr, return_index=True,
                                         return_counts=True)
        k_arr = np.arange(rr.size) - np.repeat(starts, counts)
        gcol = colmap[rr // 128, k_arr]
        assert (gcol >= 0).all()
        slot = gcol * 128 + (rr % 128)

        idx_flat = rng.integers(0, NQUAD,
                                size=total_cols * 128).astype(np.int16)
        vals4 = np.zeros((total_cols * 128, 4), np.float16)
        idx_flat[slot] = (cc2 // 4).astype(np.int16)
        vals4[slot, cc2 % 4] = vv2.astype(np.float16)

        # wrapped idx layout [128, total_cols*8] (i%16 partition, x8)
        idx_w = np.tile(
            idx_flat.reshape(total_cols * 8, 16).T, (8, 1))
        # vals [128, total_cols, 4] -> [128, total_cols*4]
        val_w = np.ascontiguousarray(
            vals4.reshape(total_cols, 128, 4).transpose(1, 0, 2)
        ).reshape(128, total_cols * 4)

        in_maps.append({
            "tab": tab,
            "idxs": np.ascontiguousarray(idx_w),
            "vals": val_w,
        })
    perms = [pc[3] for pc in per_core]
    return in_maps, windows, total_cols, perms


def _run(in_maps, windows, total_cols, trace=False):
    key = ("mod_v2", total_cols, tuple(w for w in windows))
    if key not in _cache:
        _cache[key] = _build_module(windows, total_cols)
    nc = _cache[key]
    return bass_utils.run_bass_kernel_spmd(
        nc, in_maps, core_ids=list(range(NC)), trace=trace)


def kernel(adj_row, adj_col, adj_vals, embeds, _trace=False, _return_res=False):
    adj_row = np.asarray(adj_row)
    adj_col = np.asarray(adj_col)
    adj_vals = np.asarray(adj_vals)
    embeds = np.asarray(embeds)
    in_maps, windows, total_cols, perms = _preprocess(
        adj_row, adj_col, adj_vals, embeds)
    res = _run(in_maps, windows, total_cols, trace=_trace)
    out = np.empty((N_NODES, D), np.float32)
    for m in range(NC):
        yl = np.empty((RPAD, D), np.float32)
        yl[perms[m]] = res.results[m]["y"]
        out[m * RPC:(m + 1) * RPC] = yl[:RPC]
    out = np.ascontiguousarray(out)
    if _return_res:
        return out, res
    return out



# revision 4
# speedup vs baseline: 1.3073x; 1.3073x over previous
"""SpMM (GCN layer) kernel v2 for 8 TRN2 NeuronCores.

out[i] = sum_{e: row[e]==i} vals[e] * embeds[col[e]]     (N=100000, E=3.2M, d=32)

v2 vs baseline: the bottleneck is SWDGE descriptor generation on GpSimd
(~4ns/descriptor, 1 desc per gathered slot), so the design minimizes
slot count:
  - Single logical chunk: the table is packed as fp16 QUADS (4 nodes
    = 128 fp16 = 256B payload per descriptor), so a 15-bit index covers
    all 100000/4 = 25000 quads.  Per-edge sub-node selection is folded
    into the vals multiply (vals4[slot, s] = val if col%4==s else 0).
  - Exact-K slots: per core, rows are sorted by degree (host-side
    permutation, undone for free on output) and each 128-row q-block
    gets K_q = max degree in block slots per row.  ~404K slots/core vs
    603K padded + overflow batches in the baseline; no overflow path.
  - The window schedule (gather instruction sizes / reduce shapes) is
    shared across cores: K_q = max over the 8 cores at each block rank.
"""

import sys

if "/opt/trn_rl_repo" not in sys.path:
    sys.path.insert(0, "/opt/trn_rl_repo")

import numpy as np

import concourse.bass as bass
import concourse.tile as tile
from concourse import bacc, mybir, bass_utils
from concourse import ap_utils
from concourse.bass import round_up_to_multiple, exact_div

# ---------------- problem geometry (hardcoded) ----------------
N_NODES = 100000
N_EDGES = 3200000
D = 32
NC = 8
RPC = N_NODES // NC            # rows per core = 12500
RPAD = 12544                   # 98 * 128
QROWS = RPAD // 128            # 98
NQUAD = N_NODES // 4           # 25000 quads, int16-indexable
PAYLOAD = 128                  # fp16 elems per gathered quad (256B)
TSTRIDE = 256                  # fp16 elems per table row (512B, 2x payload)
TCOL_MAX = 128                 # max gather-window columns
NUM_Q = 4                      # swdge queues used (desc-gen parallelizes per queue)
SINGLE_PACKET = False

_cache = {}


def _dma_gather_raw(gp, out_ap, in_ap, idxs_ap, num_idxs, num_idxs_reg,
                    elem_size, elem_step, queue_num=0, single_packet=False):
    """dma_gather minus the blanket elem_size_bytes%256 assert (payload
    size is free; only the descriptor stride needs 256B granularity)."""
    assert idxs_ap.dtype == mybir.dt.int16
    assert in_ap.dtype == out_ap.dtype
    assert in_ap.space == bass.MemorySpace.DRAM
    assert idxs_ap.space == bass.MemorySpace.SBUF
    assert out_ap.space == bass.MemorySpace.SBUF
    assert ap_utils.ap_is_contiguous(in_ap.ap[1:])
    assert ap_utils.ap_is_contiguous(out_ap.ap[1:])
    assert ap_utils.ap_is_contiguous(idxs_ap.ap[1:])
    assert in_ap.ap[-1][1] == out_ap.ap[-1][1] == elem_size
    assert out_ap.ap[0][1] * out_ap.ap[1][1] == round_up_to_multiple(num_idxs, 128)
    assert in_ap.ap[0][0] == elem_step
    stride_bytes = elem_step * mybir.dt.size(in_ap.dtype)
    stride_bytes_256 = exact_div(stride_bytes, 256)
    assert stride_bytes_256 < 256
    _in_ap = gp.lower_ap_dma(in_ap, for_custom_bir_dma=True)
    _idxs_ap = gp.lower_ap(idxs_ap)
    _out_ap = gp.lower_ap(out_ap)
    return gp.add_instruction(
        mybir.InstDMAGatherAnt(
            name=gp.bass.get_next_instruction_name(),
            ins=[*_in_ap, _idxs_ap, gp.lower_val_access(gp.to_reg(num_idxs_reg))],
            outs=[_out_ap],
            transpose=False,
            num_idxs=num_idxs,
            elem_size=elem_size,
            stride_bytes_256=stride_bytes_256,
            gen_mode=0,
            single_packet=single_packet,
            queue_num=queue_num,
            sbuf_tokens_per_rank=0,
            sbuf_free_dim_per_rank=0,
            sbuf_free_dim_pad_per_rank=0,
            sbuf_byte_offset=0,
        )
    )


def _schedule(K_q):
    """Shared window schedule from the per-rank max-K profile.
    Returns list of (qa, qe, k0, k1, accumulate) and total_cols."""
    windows = []
    q = 0
    while q < QROWS:
        K = int(K_q[q])
        if K == 0:
            break
        qb = q
        while qb < QROWS and K_q[qb] == K:
            qb += 1
        if K > TCOL_MAX:
            for qq in range(q, qb):
                k0 = 0
                while k0 < K:
                    k1 = min(k0 + TCOL_MAX, K)
                    windows.append((qq, qq + 1, k0, k1, k0 > 0))
                    k0 = k1
        else:
            nq_max = max(1, TCOL_MAX // K)
            qa = q
            while qa < qb:
                qe = min(qa + nq_max, qb)
                windows.append((qa, qe, 0, K, False))
                qa = qe
        q = qb
    total_cols = sum((qe - qa) * (k1 - k0) for qa, qe, k0, k1, _ in windows)
    return windows, total_cols


def _build_module(windows, total_cols):
    nc = bacc.Bacc("TRN2", target_bir_lowering=False, num_swdge_queues=NUM_Q)
    f32, f16, i16 = mybir.dt.float32, mybir.dt.float16, mybir.dt.int16

    tab = nc.dram_tensor("tab", [NQUAD, TSTRIDE], f16, kind="ExternalInput")
    idxs = nc.dram_tensor("idxs", [128, total_cols * 8], i16, kind="ExternalInput")
    vals = nc.dram_tensor("vals", [128, total_cols * 4], f16, kind="ExternalInput")
    y = nc.dram_tensor("y", [RPAD, D], f32, kind="ExternalOutput")

    with tile.TileContext(nc) as tc:
        with tc.tile_pool(name="acc", bufs=1) as accp, \
             tc.tile_pool(name="work", bufs=4) as wp, \
             nc.allow_low_precision(reason="f16 4-term sub-node fold; "
                                    "final k-reduce accumulates in f32"):
            out_acc = accp.tile([128, QROWS * D], f32)
            nc.vector.memset(out_acc[:], 0.0)

            # warmup: a tiny gather triggers the Q7 library load + ring
            # setup while the first real idx tile is still loading
            wi_t = accp.tile([128, 8], i16)
            wg_t = accp.tile([128, PAYLOAD], f16)
            nc.vector.memset(wi_t[:], 0)
            _dma_gather_raw(
                nc.gpsimd,
                wg_t[:].rearrange("p (c e) -> p c e", e=PAYLOAD),
                tab[:, :PAYLOAD], wi_t[:],
                num_idxs=128, num_idxs_reg=128,
                elem_size=PAYLOAD, elem_step=TSTRIDE,
                queue_num=0, single_packet=SINGLE_PACKET)

            def write_y(qa, qe):
                nc.sync.dma_start(
                    out=y[qa * 128:qe * 128, :].rearrange(
                        "(q p) d -> p q d", p=128),
                    in_=out_acc[:, qa * D:qe * D].rearrange(
                        "p (q d) -> p q d", d=D))

            # a window is the final writer of its q-range iff no later
            # window touches the same q-blocks
            final_writer = []
            for w, (qa, qe, _, _, _) in enumerate(windows):
                later = windows[w + 1:]
                final_writer.append(
                    all(qe <= la or qa >= le for la, le, _, _, _ in later))

            off = 0
            for w, (qa, qe, k0, k1, acc_flag) in enumerate(windows):
                nq = qe - qa
                kk = k1 - k0
                cols = nq * kk
                nidx = cols * 128
                idx_t = wp.tile([128, cols * 8], i16, tag="idx")
                val_t = wp.tile([128, cols * 4], f16, tag="val")
                g_t = wp.tile([128, cols * PAYLOAD], f16, tag="g")
                nc.sync.dma_start(
                    out=idx_t[:], in_=idxs[:, off * 8:(off + cols) * 8])
                nc.scalar.dma_start(
                    out=val_t[:], in_=vals[:, off * 4:(off + cols) * 4])
                # split the gather across all SWDGE queues; desc-gen for the
                # queues runs concurrently on the Q7 (one burst serves all)
                base = cols // NUM_Q
                rem = cols % NUM_Q
                c0 = 0
                for qn in range(NUM_Q):
                    cw = base + (1 if qn < rem else 0)
                    if cw == 0:
                        continue
                    c1 = c0 + cw
                    _dma_gather_raw(
                        nc.gpsimd,
                        g_t[:, c0 * PAYLOAD:c1 * PAYLOAD].rearrange(
                            "p (c e) -> p c e", e=PAYLOAD),
                        tab[:, :PAYLOAD], idx_t[:, c0 * 8:c1 * 8],
                        num_idxs=cw * 128, num_idxs_reg=cw * 128,
                        elem_size=PAYLOAD, elem_step=TSTRIDE,
                        queue_num=qn, single_packet=SINGLE_PACKET)
                    c0 = c1
                # scale by vals4 (selects the sub-node); table quads are
                # interleaved [d, s] so the innermost axis is contiguous
                # (s, step 1) on every operand -> DVE 2x_1P mode.  The
                # s-fold runs as tensor_tensor halves (tensor_reduce over
                # the packed axis cannot use 2x).
                gv = g_t[:].rearrange("p (c d s) -> p c d s", s=4, d=D)
                vb = val_t[:].rearrange("p (c s) -> p c s", s=4) \
                    .unsqueeze(2).broadcast_to((128, cols, D, 4))
                nc.vector.tensor_tensor(out=gv, in0=gv, in1=vb,
                                        op=mybir.AluOpType.mult)
                nc.vector.tensor_tensor(
                    out=gv[:, :, :, 0:2], in0=gv[:, :, :, 0:2],
                    in1=gv[:, :, :, 2:4], op=mybir.AluOpType.add)
                r1_t = wp.tile([128, cols * D], f16, tag="r1")
                nc.vector.tensor_tensor(
                    out=r1_t[:].rearrange("p (c d) -> p c d", d=D),
                    in0=gv[:, :, :, 0], in1=gv[:, :, :, 1],
                    op=mybir.AluOpType.add)
                # reduce over k -> [p, nq, d]
                rin = r1_t[:].rearrange("p (q k d) -> p q d k", k=kk, d=D)
                if not acc_flag:
                    nc.vector.tensor_reduce(
                        out=out_acc[:, qa * D:qe * D].rearrange(
                            "p (q d) -> p q d", d=D),
                        in_=rin, axis=mybir.AxisListType.X,
                        op=mybir.AluOpType.add)
                else:
                    red_t = wp.tile([128, nq * D], f32, tag="red")
                    nc.vector.tensor_reduce(
                        out=red_t[:].rearrange("p (q d) -> p q d", d=D),
                        in_=rin, axis=mybir.AxisListType.X,
                        op=mybir.AluOpType.add)
                    nc.vector.tensor_tensor(
                        out=out_acc[:, qa * D:qe * D],
                        in0=out_acc[:, qa * D:qe * D],
                        in1=red_t[:], op=mybir.AluOpType.add)
                if final_writer[w]:
                    write_y(qa, qe)
                off += cols

            covered = set()
            for qa, qe, _, _, _ in windows:
                covered.update(range(qa, qe))
            q = 0
            while q < QROWS:
                if q in covered:
                    q += 1
                    continue
                qe = q
                while qe < QROWS and qe not in covered:
                    qe += 1
                write_y(q, qe)
                q = qe

    nc.finalize()
    return nc


def _preprocess(adj_row, adj_col, adj_vals, embeds):
    order = np.argsort(adj_row, kind="stable")
    rows = np.ascontiguousarray(adj_row[order]).astype(np.int64)
    cols = np.ascontiguousarray(adj_col[order]).astype(np.int64)
    vals = np.ascontiguousarray(adj_vals[order]).astype(np.float32)
    bounds = np.searchsorted(rows, np.arange(NC + 1) * RPC)

    # fp16 quad table, 512B row stride (payload in first 256B)
    tab = np.zeros((NQUAD, TSTRIDE), np.float16)
    tab[:, :PAYLOAD] = (embeds.astype(np.float16)
                        .reshape(NQUAD, 4, D)
                        .transpose(0, 2, 1)
                        .reshape(NQUAD, PAYLOAD))

    # per-core degree-sorted rows; shared K profile
    per_core = []
    K_prof = np.zeros(QROWS, np.int64)
    for m in range(NC):
        s, e = bounds[m], bounds[m + 1]
        rl = rows[s:e] - m * RPC
        deg = np.bincount(rl, minlength=RPAD)
        perm = np.argsort(-deg, kind="stable")   # perm[rank] = local row
        rank_of = np.empty(RPAD, np.int64)
        rank_of[perm] = np.arange(RPAD)
        K_prof = np.maximum(K_prof,
                            deg[perm].reshape(QROWS, 128).max(axis=1))
        per_core.append((s, e, rl, perm, rank_of))

    windows, total_cols = _schedule(K_prof)

    # map (q, k) -> global column
    colmap = np.full((QROWS, int(K_prof.max())), -1, np.int64)
    off = 0
    for qa, qe, k0, k1, _ in windows:
        kk = k1 - k0
        for qq in range(qa, qe):
            colmap[qq, k0:k1] = off + (qq - qa) * kk + np.arange(kk)
        off += (qe - qa) * kk
    assert off == total_cols

    rng = np.random.default_rng(12345)
    in_maps = []
    for m in range(NC):
        s, e, rl, perm, rank_of = per_core[m]
        cc = cols[s:e]
        vv = vals[s:e]
        rr = rank_of[rl]
        o2 = np.argsort(rr, kind="stable")
        rr, cc2, vv2 = rr[o2], cc[o2], vv[o2]
        uniq, starts, counts = np.unique(rr, return_index=True,
                                         return_counts=True)
        k_arr = np.arange(rr.size) - np.repeat(starts, counts)
        gcol = colmap[rr // 128, k_arr]
        assert (gcol >= 0).all()
        slot = gcol * 128 + (rr % 128)

        idx_flat = rng.integers(0, NQUAD,
                                size=total_cols * 128).astype(np.int16)
        vals4 = np.zeros((total_cols * 128, 4), np.float16)
        idx_flat[slot] = (cc2 // 4).astype(np.int16)
        vals4[slot, cc2 % 4] = vv2.astype(np.float16)

        # wrapped idx layout [128, total_cols*8] (i%16 partition, x8)
        idx_w = np.tile(
            idx_flat.reshape(total_cols * 8, 16).T, (8, 1))
        # vals [128, total_cols, 4] -> [128, total_cols*4]
        val_w = np.ascontiguousarray(
            vals4.reshape(total_cols, 128, 4).transpose(1, 0, 2)
        ).reshape(128, total_cols * 4)

        in_maps.append({
            "tab": tab,
            "idxs": np.ascontiguousarray(idx_w),
            "vals": val_w,
        })
    perms = [pc[3] for pc in per_core]
    return in_maps, windows, total_cols, perms


def _run(in_maps, windows, total_cols, trace=False):
    key = ("mod_v2", total_cols, tuple(w for w in windows))
    if key not in _cache:
        _cache[key] = _build_module(windows, total_cols)
    nc = _cache[key]
    return bass_utils.run_bass_kernel_spmd(
        nc, in_maps, core_ids=list(range(NC)), trace=trace)


def kernel(adj_row, adj_col, adj_vals, embeds, _trace=False, _return_res=False):
    adj_row = np.asarray(adj_row)
    adj_col = np.asarray(adj_col)
    adj_vals = np.asarray(adj_vals)
    embeds = np.asarray(embeds)
    in_maps, windows, total_cols, perms = _preprocess(
        adj_row, adj_col, adj_vals, embeds)
    res = _run(in_maps, windows, total_cols, trace=_trace)
    out = np.empty((N_NODES, D), np.float32)
    for m in range(NC):
        yl = np.empty((RPAD, D), np.float32)
        yl[perms[m]] = res.results[m]["y"]
        out[m * RPC:(m + 1) * RPC] = yl[:RPC]
    out = np.ascontiguousarray(out)
    if _return_res:
        return out, res
    return out

